# revision 22
# baseline (speedup 1.0000x reference)
"""Trainium2 Bass kernel for MesoBranched GNN message passing.

Self-contained: hardcodes problem shapes (N=50000, E=500000, F_IN=256, H=128,
T=2, G=8), shards across 8 NeuronCores internally.

Distribution: nodes split equally across cores (6250/core, padded to 6656 =
52 blocks x 128); edges assigned to the core owning dst, sorted by dst and
packed into node-aligned 128-edge chunks (K chunks per node block). EdgeConv
first linear is decomposed into node-space matmuls U=x@(W1a-W1b), V=x@W1b so
the edge-space work is gather(U[dst])+gather(V[src]), BN+ReLU, one HxH matmul,
and an indicator-matmul scatter accumulated in PSUM. BN statistics are exact:
batch stats over all real edges via node-space weighted sums + a PSUM-
accumulated cross term, and bn_stats over the second linear, AllReduced
across cores. V is AllGathered per layer.
"""
import math
import numpy as np
import ml_dtypes

BF16 = ml_dtypes.bfloat16

# problem constants
N, E, F_IN, H, T, G = 50000, 500000, 256, 128, 2, 8
NC = 8
NPC = N // NC            # 6250 real nodes per core
NBLK = 52                # node blocks per core
NLOC = NBLK * 128        # 6656 node slots per core
NVT = NC * NLOC          # V table rows
CH = 128                 # edges per chunk
BN_EPS = 1e-5
PADROW = NPC + 4         # a guaranteed-zero local node slot used by pad edges

_CACHE = {}


# ---------------------------------------------------------------- host prep
def _prep(xfeat, edge_index, batch):
    src = np.asarray(edge_index[0], dtype=np.int64)
    dst = np.asarray(edge_index[1], dtype=np.int64)
    ecore = dst // NPC
    per_core = []
    K = 1
    for c in range(NC):
        sel = np.nonzero(ecore == c)[0]
        dl = (dst[sel] - c * NPC).astype(np.int64)
        order = np.argsort(dl, kind="stable")
        sel = sel[order]
        dl = dl[order]
        deg = np.bincount(dl, minlength=NLOC).astype(np.int64)
        bdeg = deg.reshape(NBLK, 128).sum(1)
        K = max(K, int(math.ceil(bdeg.max() / CH)))
        per_core.append((sel, dl, deg, bdeg))

    out = []
    for c in range(NC):
        sel, dl, deg, bdeg = per_core[c]
        slots = NBLK * K * CH
        idxU = np.full(slots, PADROW, dtype=np.int32)
        idxV = np.full(slots, c * NLOC + PADROW, dtype=np.int32)
        dstrel = np.full(slots, -1.0, dtype=np.float32)
        # edges of block b occupy the first bdeg[b] slots of its K*CH window
        epos = 0
        for b in range(NBLK):
            nb = int(bdeg[b])
            if nb == 0:
                continue
            s0 = b * K * CH
            eb = sel[epos:epos + nb]
            dlb = dl[epos:epos + nb]
            epos += nb
            idxU[s0:s0 + nb] = dlb.astype(np.int32)
            sc = src[eb]
            idxV[s0:s0 + nb] = (sc // NPC) * NLOC + (sc % NPC)
            dstrel[s0:s0 + nb] = (dlb - b * 128).astype(np.float32)
        cnt = np.maximum(deg, 1).astype(np.float32)
        cntinv = (1.0 / cnt).reshape(NBLK, 128).T.copy()        # [128, NBLK]
        indeg_nm = deg.astype(np.float32).reshape(NBLK, 128).T.copy()
        odg = np.bincount(src, minlength=N)[c * NPC:(c + 1) * NPC]
        outdeg = np.zeros(NLOC, np.float32)
        outdeg[:NPC] = odg
        outdeg_nm = outdeg.reshape(NBLK, 128).T.copy()
        xfT = np.zeros((F_IN, NLOC), np.float32)
        xfT[:, :NPC] = np.asarray(xfeat[c * NPC:(c + 1) * NPC]).T
        nchunk = NBLK * K
        out.append(dict(
            xfT=xfT,
            idxU=idxU.reshape(nchunk, 128).T.copy(),            # [128, nchunk]
            idxV=idxV.reshape(nchunk, 128).T.copy(),
            dstrel=dstrel.reshape(nchunk, 128).T.copy(),
            cntinv=cntinv,
            indeg_nm=indeg_nm.astype(BF16),
            outdeg_nm=outdeg_nm.astype(BF16),
            npadv=np.full((128, 1), float(slots - len(sel)), np.float32),
        ))
    return out, K


def _weights(inp):
    w = {}
    w["fhW1"] = np.asarray(inp["fh_W1"], np.float32)            # [256,128]
    w["fhW2"] = np.asarray(inp["fh_W2"], np.float32)            # [128,128]
    for l in range(2):
        W1 = np.asarray(inp["ec_W1"][l], np.float32)            # [256,128]
        w[f"A{l}"] = W1[:H] - W1[H:]
        w[f"B{l}"] = W1[H:]
        w[f"W2f{l}"] = np.asarray(inp["ec_W2"][l], np.float32)
        w[f"W2b{l}"] = w[f"W2f{l}"].astype(BF16)
    for l in range(3):
        w[f"linW{l}"] = np.asarray(inp["lin_W"][l], np.float32)  # [128,2]
    w["lbsum"] = np.asarray(inp["lin_b"], np.float32).sum(0).reshape(T, 1)
    for nm in ["fh_g1", "fh_be1", "fh_g2", "fh_be2"]:
        w[nm] = np.asarray(inp[nm], np.float32).reshape(H, 1)
    for l in range(2):
        for nm in ["ec_g1", "ec_be1", "ec_g2", "ec_be2"]:
            w[f"{nm}_{l}"] = np.asarray(inp[nm][l], np.float32).reshape(H, 1)
    w["iota128"] = np.tile(np.arange(128, dtype=np.float32), (128, 1))
    w["ident128"] = np.eye(128, dtype=np.float32)
    w["onesrow"] = np.ones((1, 128), np.float32)
    return w


# ---------------------------------------------------------------- bass build
def _build(K):
    import concourse.bacc as bacc
    import concourse.tile as tile
    from concourse import bass, mybir

    F32, BF, I32 = mybir.dt.float32, mybir.dt.bfloat16, mybir.dt.int32
    AF = mybir.ActivationFunctionType
    OP = mybir.AluOpType
    NCHUNK = NBLK * K
    SLOTS = NCHUNK * CH
    NG = NCHUNK // 4          # pass-2 groups of 512

    nc = bacc.Bacc("TRN2", target_bir_lowering=False, debug=False,
                   enable_asserts=False, num_devices=NC)

    def inp(name, shape, dt):
        return nc.dram_tensor(name, shape, dt, kind="ExternalInput").ap()

    xfT_d = inp("xfT", [F_IN, NLOC], F32)
    idxU_d = inp("idxU", [128, NCHUNK], I32)
    idxV_d = inp("idxV", [128, NCHUNK], I32)
    dstrel_d = inp("dstrel", [128, NCHUNK], F32)
    cntinv_d = inp("cntinv", [128, NBLK], F32)
    indeg_d = inp("indeg_nm", [128, NBLK], BF)
    outdeg_d = inp("outdeg_nm", [128, NBLK], BF)
    npadv_d = inp("npadv", [128, 1], F32)
    fhW1_d = inp("fhW1", [F_IN, H], F32)
    fhW2_d = inp("fhW2", [H, H], F32)
    wl = {}
    for l in range(2):
        for nm in ["A", "B", "W2f"]:
            wl[f"{nm}{l}"] = inp(f"{nm}{l}", [H, H], F32)
        wl[f"W2b{l}"] = inp(f"W2b{l}", [H, H], BF)
        for nm in ["ec_g1", "ec_be1", "ec_g2", "ec_be2"]:
            wl[f"{nm}_{l}"] = inp(f"{nm}_{l}", [H, 1], F32)
    lin_d = [inp(f"linW{l}", [H, T], F32) for l in range(3)]
    lbsum_d = inp("lbsum", [T, 1], F32)
    gb = {nm: inp(nm, [H, 1], F32)
          for nm in ["fh_g1", "fh_be1", "fh_g2", "fh_be2"]}
    iota_d = inp("iota128", [128, 128], F32)
    ident_d = inp("ident128", [128, 128], F32)
    ones_d = inp("onesrow", [1, 128], F32)
    Z_out = nc.dram_tensor("Z_out", [T, NLOC], F32, kind="ExternalOutput").ap()

    # internal DRAM
    U_nm_d = nc.dram_tensor("U_nm", [NLOC, H], BF).ap()
    Vloc_d = nc.dram_tensor("Vloc", [NLOC, H], BF).ap()
    Vfull_d = [nc.dram_tensor(f"Vfull{l}", [NVT, H], BF,
                              addr_space="Shared").ap() for l in range(2)]
    r_d = nc.dram_tensor("r_ed", [SLOTS, H], BF).ap()
    h_d = nc.dram_tensor("h_ed", [H, SLOTS], BF).ap()
    AR_W = [2, 2, 5, 2, 5, 2]
    arI = [nc.dram_tensor(f"arI{i}", [128, w], F32).ap()
           for i, w in enumerate(AR_W)]
    arO = [nc.dram_tensor(f"arO{i}", [128, w], F32, addr_space="Shared").ap()
           for i, w in enumerate(AR_W)]
    _arn = [0]

    with tile.TileContext(nc) as tc:
      with tc.tile_pool(name="pers", bufs=1) as pp, \
           tc.tile_pool(name="wk", bufs=3) as wk, \
           tc.tile_pool(name="vec", bufs=1) as vec, \
           tc.tile_pool(name="ps", bufs=2, space="PSUM") as ps, \
           tc.tile_pool(name="psP", bufs=1, space="PSUM") as psP:

        # ---- persistent constants
        iota_t = pp.tile([128, 128], F32, tag="iota")
        nc.sync.dma_start(iota_t[:], iota_d[:])
        ident_t = pp.tile([128, 128], F32, tag="ident")
        nc.sync.dma_start(ident_t[:], ident_d[:])
        ones_t = pp.tile([1, 128], F32, tag="ones")
        nc.sync.dma_start(ones_t[:], ones_d[:])
        cnti_t = pp.tile([128, NBLK], F32, tag="cnti")
        nc.sync.dma_start(cnti_t[:], cntinv_d[:])
        ideg_t = pp.tile([128, NBLK], BF, tag="ideg")
        nc.sync.dma_start(ideg_t[:], indeg_d[:])
        odeg_t = pp.tile([128, NBLK], BF, tag="odeg")
        nc.sync.dma_start(odeg_t[:], outdeg_d[:])
        npad_t = pp.tile([128, 1], F32, tag="npad")
        nc.sync.dma_start(npad_t[:], npadv_d[:])

        # small helper ops
        V = nc.vector
        S = nc.scalar

        def vt(tag, w=1, dt=F32, p=128):
            return vec.tile([p, w], dt, tag=tag, name=tag)

        def allreduce(pack_t, width, tag):
            """AllReduce-add a [128,width] f32 sbuf tile; returns result tile."""
            i = _arn[0]; _arn[0] += 1
            assert AR_W[i] == width
            nc.sync.dma_start(arI[i][:], pack_t[:, :width])
            nc.gpsimd.collective_compute(
                "AllReduce", OP.add, replica_groups=[list(range(NC))],
                ins=[arI[i][:]], outs=[arO[i][:]])
            r = vt(f"arres{tag}", width)
            nc.sync.dma_start(r[:, :width], arO[i][:])
            return r

        def bn_apply_vecs(sum_t, sum2_t, g_t, be_t, count, tag):
            """From global sums -> s,t vectors: s=g*rsqrt(var+eps),
            t=be-s*mean."""
            mean = vt(f"mean{tag}")
            V.tensor_scalar(mean[:], sum_t[:], 1.0 / count, None, OP.mult)
            ex2 = vt(f"ex2{tag}")
            V.tensor_scalar(ex2[:], sum2_t[:], 1.0 / count, None, OP.mult)
            m2 = vt(f"m2{tag}")
            V.tensor_mul(m2[:], mean[:], mean[:])
            var = vt(f"var{tag}")
            V.tensor_sub(var[:], ex2[:], m2[:])
            V.tensor_scalar(var[:], var[:], BN_EPS, None, OP.add)
            rv = vt(f"rv{tag}")
            V.reciprocal(rv[:], var[:])
            rs = vt(f"rs{tag}")
            S.sqrt(rs[:], rv[:])
            s_t = vt(f"s{tag}")
            V.tensor_mul(s_t[:], g_t[:], rs[:])
            st = vt(f"st{tag}")
            V.tensor_mul(st[:], s_t[:], mean[:])
            t_t = vt(f"t{tag}")
            V.tensor_sub(t_t[:], be_t[:], st[:])
            return s_t, t_t

        # ================= stage 0: first_h =================
        xf0 = pp.tile([128, NLOC], F32, tag="bigA")
        nc.sync.dma_start(xf0[:], xfT_d[0:128, :])
        xf1 = pp.tile([128, NLOC], F32, tag="bigB")
        nc.sync.dma_start(xf1[:], xfT_d[128:256, :])
        w1a = pp.tile([128, H], F32, tag="w1a")
        nc.sync.dma_start(w1a[:], fhW1_d[0:128, :])
        w1b = pp.tile([128, H], F32, tag="w1b")
        nc.sync.dma_start(w1b[:], fhW1_d[128:256, :])
        w2fh = pp.tile([128, H], F32, tag="w2fh")
        nc.sync.dma_start(w2fh[:], fhW2_d[:])
        gbt = {}
        for nm in gb:
            gbt[nm] = vt(f"g_{nm}")
            nc.sync.dma_start(gbt[nm][:], gb[nm][:])

        NCH0 = NLOC // 512  # 13
        p_sb = pp.tile([128, NLOC], F32, tag="bigC")
        bn0 = vt("bn0", NCH0 * 6)
        for j in range(NCH0):
            sl = slice(j * 512, (j + 1) * 512)
            pps = ps.tile([128, 512], F32, tag="big512")
            nc.tensor.matmul(pps[:], lhsT=w1a[:], rhs=xf0[:, sl],
                             start=True, stop=False)
            nc.tensor.matmul(pps[:], lhsT=w1b[:], rhs=xf1[:, sl],
                             start=False, stop=True)
            V.bn_stats(bn0[:, j * 6:(j + 1) * 6], pps[:])
            S.copy(p_sb[:, sl], pps[:])
        mv0 = vt("mv0", 2)
        V.bn_aggr(mv0[:], bn0[:])
        # sums over slots (pads contribute 0): sum = mean*NLOC,
        # sum2 = (var+mean^2)*NLOC
        pk0 = vt("pk0", 2)
        V.tensor_scalar(pk0[:, 0:1], mv0[:, 0:1], float(NLOC), None, OP.mult)
        t0 = vt("t0_")
        V.tensor_mul(t0[:], mv0[:, 0:1], mv0[:, 0:1])
        V.tensor_add(t0[:], t0[:], mv0[:, 1:2])
        V.tensor_scalar(pk0[:, 1:2], t0[:], float(NLOC), None, OP.mult)
        ar0 = allreduce(pk0, 2, "fc1")
        s_fc1, t_fc1 = bn_apply_vecs(ar0[:, 0:1], ar0[:, 1:2],
                                     gbt["fh_g1"], gbt["fh_be1"], N, "fc1")
        h1 = pp.tile([128, NLOC], F32, tag="bigC2")
        S.activation(h1[:], p_sb[:], AF.Relu, bias=t_fc1[:], scale=s_fc1[:])

        # fc2
        q_sb = pp.tile([128, NLOC], F32, tag="bigB")  # reuses xf1 slot
        bn1v = vt("bn1v", NCH0 * 6)
        for j in range(NCH0):
            sl = slice(j * 512, (j + 1) * 512)
            pps = ps.tile([128, 512], F32, tag="big512")
            nc.tensor.matmul(pps[:], lhsT=w2fh[:], rhs=h1[:, sl],
                             start=True, stop=True)
            V.bn_stats(bn1v[:, j * 6:(j + 1) * 6], pps[:])
            S.copy(q_sb[:, sl], pps[:])
        mv1 = vt("mv1", 2)
        V.bn_aggr(mv1[:], bn1v[:])
        # pad correction: h1pad = Relu(t_fc1); qpad = W2fh^T h1pad
        h1p = vt("h1p")
        S.activation(h1p[:], t_fc1[:], AF.Relu)
        qp_ps = psP.tile([128, 1], F32, tag="aggq")
        nc.tensor.matmul(qp_ps[:], lhsT=w2fh[:], rhs=h1p[:],
                         start=True, stop=True)
        qpad = vt("qpad")
        S.copy(qpad[:], qp_ps[:])
        qpad2 = vt("qpad2")
        V.tensor_mul(qpad2[:], qpad[:], qpad[:])
        npadN = float(NLOC - NPC)
        pk1 = vt("pk1", 2)
        V.tensor_scalar(pk1[:, 0:1], mv1[:, 0:1], float(NLOC), None, OP.mult)
        t1_ = vt("t1_")
        V.tensor_mul(t1_[:], mv1[:, 0:1], mv1[:, 0:1])
        V.tensor_add(t1_[:], t1_[:], mv1[:, 1:2])
        V.tensor_scalar(t1_[:], t1_[:], float(NLOC), None, OP.mult)
        c0 = vt("c0_")
        V.tensor_scalar(c0[:], qpad[:], npadN, None, OP.mult)
        V.tensor_sub(pk1[:, 0:1], pk1[:, 0:1], c0[:])
        V.tensor_scalar(c0[:], qpad2[:], npadN, None, OP.mult)
        V.tensor_sub(pk1[:, 1:2], t1_[:], c0[:])
        ar1 = allreduce(pk1, 2, "fc2")
        s_fc2, t_fc2 = bn_apply_vecs(ar1[:, 0:1], ar1[:, 1:2],
                                     gbt["fh_g2"], gbt["fh_be2"], N, "fc2")
        x_fm = pp.tile([128, NLOC], F32, tag="bigA")  # reuses xf0 slot
        S.activation(x_fm[:], q_sb[:], AF.Relu, bias=t_fc2[:], scale=s_fc2[:])
        V.memset(x_fm[:, NPC:NLOC], 0.0)

        # Z accumulator
        Zacc = pp.tile([T, NLOC], F32, tag="Zacc")
        linT = []
        for l in range(3):
            lt = pp.tile([128, T], F32, tag=f"lin{l}")
            nc.sync.dma_start(lt[:], lin_d[l][:])
            linT.append(lt)

        def z_update(xt, l, first):
            for j in range(NCH0):
                sl = slice(j * 512, (j + 1) * 512)
                zp = ps.tile([T, 512], F32, tag="big512")
                nc.tensor.matmul(zp[:], lhsT=linT[l][:], rhs=xt[:, sl],
                                 start=True, stop=True)
                if first:
                    S.copy(Zacc[:, sl], zp[:])
                else:
                    V.tensor_add(Zacc[:, sl], Zacc[:, sl], zp[:])

        z_update(x_fm, 0, True)

        # ================= EdgeConv layers =================
        x_cur = x_fm
        for l in range(2):
            A_t = pp.tile([128, H], F32, tag="A_t")
            nc.sync.dma_start(A_t[:], wl[f"A{l}"][:])
            B_t = pp.tile([128, H], F32, tag="B_t")
            nc.sync.dma_start(B_t[:], wl[f"B{l}"][:])
            W2b_t = pp.tile([128, H], BF, tag="W2b_t")
            nc.sync.dma_start(W2b_t[:], wl[f"W2b{l}"][:])
            W2f_t = pp.tile([128, H], F32, tag="W2f_t")
            nc.sync.dma_start(W2f_t[:], wl[f"W2f{l}"][:])
            egb = {}
            for nm in ["ec_g1", "ec_be1", "ec_g2", "ec_be2"]:
                egb[nm] = vt(f"l{l}{nm}")
                nc.sync.dma_start(egb[nm][:], wl[f"{nm}_{l}"][:])

            # U,V node-space; write U_nm / Vloc node-major (bf16)
            U_sb = pp.tile([128, NBLK, 128], BF, tag="bfA")
            V_sb = pp.tile([128, NBLK, 128], BF, tag="bfB")
            for (Wt, dest) in ((A_t, U_sb), (B_t, V_sb)):
                for b in range(NBLK):
                    sl = slice(b * 128, (b + 1) * 128)
                    up = ps.tile([128, 128], F32, tag="sm128")
                    nc.tensor.matmul(up[:], lhsT=Wt[:], rhs=x_cur[:, sl],
                                     start=True, stop=True)
                    uf = wk.tile([128, 128], F32, tag="uvf")
                    S.copy(uf[:], up[:])
                    tp = ps.tile([128, 128], F32, tag="sm128")
                    nc.tensor.transpose(tp[:], uf[:], ident_t[:])
                    S.copy(dest[:, b, :], tp[:])
            nc.sync.dma_start(
                U_nm_d[:].rearrange("(b p) f -> p b f", p=128), U_sb[:])
            nc.sync.dma_start(
                Vloc_d[:].rearrange("(b p) f -> p b f", p=128), V_sb[:])
            nc.gpsimd.collective_compute(
                "AllGather", mybir.AluOpType.bypass,
                replica_groups=[list(range(NC))],
                ins=[Vloc_d[:]], outs=[Vfull_d[l][:]])

            # ---- pass 1: gathers, M1 cross-term, r store
            M1_ps = psP.tile([128, 128], F32, tag="M1")
            for c in range(NCHUNK):
                if c % 16 == 0:
                    w16 = min(16, NCHUNK - c)
                    iu = wk.tile([128, 16], I32, tag="iu")
                    nc.sync.dma_start(iu[:, :w16], idxU_d[:, c:c + w16])
                    iv = wk.tile([128, 16], I32, tag="iv")
                    nc.sync.dma_start(iv[:, :w16], idxV_d[:, c:c + w16])
                j = c % 16
                ug = wk.tile([128, 128], BF, tag="ug")
                nc.gpsimd.indirect_dma_start(
                    out=ug[:], out_offset=None, in_=U_nm_d[:],
                    in_offset=bass.IndirectOffsetOnAxis(ap=iu[:, j:j + 1],
                                                        axis=0))
                vg = wk.tile([128, 128], BF, tag="vg")
                nc.gpsimd.indirect_dma_start(
                    out=vg[:], out_offset=None, in_=Vfull_d[l][:],
                    in_offset=bass.IndirectOffsetOnAxis(ap=iv[:, j:j + 1],
                                                        axis=0))
                nc.tensor.matmul(M1_ps[:], lhsT=ug[:], rhs=vg[:],
                                 start=(c == 0), stop=(c == NCHUNK - 1))
                rr = wk.tile([128, 128], BF, tag="rr")
                V.tensor_add(rr[:], ug[:], vg[:])
                nc.sync.dma_start(r_d[c * 128:(c + 1) * 128, :], rr[:])

            # ---- S1 stats: weighted node sums + cross term
            wsum = psP.tile([128, 4], F32, tag="wsum")
            for col, (srct, wvec, sq) in enumerate(
                    [(U_sb, ideg_t, False), (V_sb, odeg_t, False),
                     (U_sb, ideg_t, True), (V_sb, odeg_t, True)]):
                for b in range(NBLK):
                    if sq:
                        uq = wk.tile([128, 128], BF, tag="uq")
                        S.square(uq[:], srct[:, b, :])
                        lh = uq[:]
                    else:
                        lh = srct[:, b, :]
                    nc.tensor.matmul(wsum[:, col:col + 1], lhsT=lh,
                                     rhs=wvec[:, b:b + 1],
                                     start=(b == 0), stop=(b == NBLK - 1))
            md = wk.tile([128, 128], F32, tag="md")
            V.tensor_mul(md[:], M1_ps[:], ident_t[:])
            cdg = vt(f"cdiag{l}")
            V.tensor_reduce(cdg[:], md[:], mybir.AxisListType.X, OP.add)
            pkS = vt(f"pkS{l}", 5)
            S.copy(pkS[:, 0:4], wsum[:])
            V.tensor_copy(pkS[:, 4:5], cdg[:])
            arS = allreduce(pkS, 5, f"S1_{l}")
            sr = vt(f"sr{l}")
            V.tensor_add(sr[:], arS[:, 0:1], arS[:, 1:2])
            sr2 = vt(f"sr2{l}")
            V.tensor_add(sr2[:], arS[:, 2:3], arS[:, 3:4])
            cc2 = vt(f"cc2{l}")
            V.tensor_scalar(cc2[:], arS[:, 4:5], 2.0, None, OP.mult)
            V.tensor_add(sr2[:], sr2[:], cc2[:])
            s1v, t1v = bn_apply_vecs(sr, sr2, egb["ec_g1"], egb["ec_be1"],
                                     E, f"e1_{l}")

            # ---- pass 2: h = relu(s1*r+t1), stats matmul, store h
            bnE = vt(f"bnE{l}", NG * 6)
            for g in range(NG):
                rT = wk.tile([128, 512], BF, tag="rT")
                nc.sync.dma_start_transpose(
                    out=rT[:], in_=r_d[g * 512:(g + 1) * 512, :])
                hh = wk.tile([128, 512], BF, tag="hh")
                S.activation(hh[:], rT[:], AF.Relu, bias=t1v[:], scale=s1v[:])
                nc.sync.dma_start(h_d[:, g * 512:(g + 1) * 512], hh[:])
                p2 = ps.tile([128, 512], F32, tag="big512")
                nc.tensor.matmul(p2[:], lhsT=W2b_t[:], rhs=hh[:],
                                 start=True, stop=True)
                V.bn_stats(bnE[:, g * 6:(g + 1) * 6], p2[:])
            mvE = vt(f"mvE{l}", 2)
            V.bn_aggr(mvE[:], bnE[:])
            # pad correction with runtime npad
            hp = vt(f"hp{l}", 1, BF)
            S.activation(hp[:], t1v[:], AF.Relu)
            ppad_ps = psP.tile([128, 1], F32, tag="aggq")
            nc.tensor.matmul(ppad_ps[:], lhsT=W2b_t[:], rhs=hp[:],
                             start=True, stop=True)
            ppad = vt(f"ppad{l}")
            S.copy(ppad[:], ppad_ps[:])
            ppad2 = vt(f"ppad2{l}")
            V.tensor_mul(ppad2[:], ppad[:], ppad[:])
            pkE = vt(f"pkE{l}", 2)
            V.tensor_scalar(pkE[:, 0:1], mvE[:, 0:1], float(SLOTS),
                            None, OP.mult)
            tE = vt(f"tE{l}")
            V.tensor_mul(tE[:], mvE[:, 0:1], mvE[:, 0:1])
            V.tensor_add(tE[:], tE[:], mvE[:, 1:2])
            V.tensor_scalar(tE[:], tE[:], float(SLOTS), None, OP.mult)
            cE = vt(f"cE{l}")
            V.tensor_mul(cE[:], npad_t[:], ppad[:])
            V.tensor_sub(pkE[:, 0:1], pkE[:, 0:1], cE[:])
            V.tensor_mul(cE[:], npad_t[:], ppad2[:])
            V.tensor_sub(pkE[:, 1:2], tE[:], cE[:])
            arE = allreduce(pkE, 2, f"S2_{l}")
            s2v, t2v = bn_apply_vecs(arE[:, 0:1], arE[:, 1:2],
                                     egb["ec_g2"], egb["ec_be2"],
                                     E, f"e2_{l}")
            # rows + scaled W2: s2row/t2row via transpose-mm; s2 bcast via
            # outer product
            s2r_ps = ps.tile([1, 128], F32, tag="sm128")
            nc.tensor.matmul(s2r_ps[:], lhsT=s2v[:], rhs=ident_t[:],
                             start=True, stop=True)
            t2r_ps = ps.tile([1, 128], F32, tag="sm128")
            nc.tensor.matmul(t2r_ps[:], lhsT=t2v[:], rhs=ident_t[:],
                             start=True, stop=True)
            t2row = vec.tile([1, 128], F32, tag=f"t2row{l}")
            S.copy(t2row[:], t2r_ps[:])
            s2row = vec.tile([1, 128], F32, tag=f"s2row{l}")
            S.copy(s2row[:], s2r_ps[:])
            s2b_ps = ps.tile([128, 128], F32, tag="sm128")
            nc.tensor.matmul(s2b_ps[:], lhsT=ones_t[:], rhs=s2row[:],
                             start=True, stop=True)
            s2b = wk.tile([128, 128], F32, tag="s2b_sb")
            S.copy(s2b[:], s2b_ps[:])
            W2s_t = pp.tile([128, H], BF, tag="W2s_t")
            V.tensor_mul(W2s_t[:], W2f_t[:], s2b[:])

            # ---- pass 3: final matmul, relu, indicator scatter
            x_new = pp.tile([128, NLOC], F32,
                            tag=("bigB" if l == 0 else "bigA"))
            for c in range(NCHUNK):
                if c % 16 == 0:
                    w16 = min(16, NCHUNK - c)
                    dr = wk.tile([128, 16], F32, tag="dr")
                    nc.sync.dma_start(dr[:, :w16], dstrel_d[:, c:c + w16])
                j = c % 16
                ht = wk.tile([128, 128], BF, tag="ht")
                nc.sync.dma_start(ht[:], h_d[:, c * 128:(c + 1) * 128])
                p2e = ps.tile([128, 128], F32, tag="sm128")
                nc.tensor.matmul(p2e[:], lhsT=ht[:], rhs=W2s_t[:],
                                 start=True, stop=False)
                nc.tensor.matmul(p2e[:], lhsT=ones_t[:], rhs=t2row[:],
                                 start=False, stop=True)
                msg = wk.tile([128, 128], BF, tag="msg")
                S.activation(msg[:], p2e[:], AF.Relu)
                ind = wk.tile([128, 128], BF, tag="ind")
                V.tensor_scalar(ind[:], iota_t[:], dr[:, j:j + 1], None,
                                OP.is_equal)
                if c % K == 0:
                    agg = psP.tile([128, 128], F32, tag="agg", name="agg")
                nc.tensor.matmul(agg[:], lhsT=ind[:], rhs=msg[:],
                                 start=(c % K == 0), stop=(c % K == K - 1))
                if c % K == K - 1:
                    b = c // K
                    xn = wk.tile([128, 128], F32, tag="xn")
                    S.mul(xn[:], agg[:], cnti_t[:, b:b + 1])
                    tpx = ps.tile([128, 128], F32, tag="sm128")
                    nc.tensor.transpose(tpx[:], xn[:], ident_t[:])
                    S.copy(x_new[:, b * 128:(b + 1) * 128], tpx[:])
            z_update(x_new, l + 1, False)
            x_cur = x_new

        # ---- finalize Z
        lbt = vec.tile([T, 1], F32, tag="lbt")
        nc.sync.dma_start(lbt[:], lbsum_d[:])
        V.tensor_scalar(Zacc[:], Zacc[:], lbt[:], None, OP.add)
        nc.sync.dma_start(Z_out[:], Zacc[:])

    nc.compile()
    return nc


# ---------------------------------------------------------------- entry
def kernel(**inputs):
    xfeat = np.asarray(inputs["xfeat"], np.float32)
    batch = np.asarray(inputs["batch"], np.int64)
    per_core, K = _prep(xfeat, inputs["edge_index"], batch)
    w = _weights(inputs)

    if K not in _CACHE:
        _CACHE[K] = _build(K)
    nc = _CACHE[K]

    in_maps = []
    for c in range(NC):
        m = dict(per_core[c])
        m.update(w)
        in_maps.append(m)
    from concourse.bass_utils import run_bass_kernel_spmd
    res = run_bass_kernel_spmd(nc, in_maps, list(range(NC)))

    Z = np.zeros((N, T), np.float32)
    for c in range(NC):
        Zc = res.results[c]["Z_out"]  # [T, NLOC]
        Z[c * NPC:(c + 1) * NPC] = Zc[:, :NPC].T
    out = np.full((G, T), -np.inf, np.float32)
    np.maximum.at(out, batch, Z)
    return out, Z
